# revision 30
# baseline (speedup 1.0000x reference)
"""CondConv2d (MoE-routed 3x3 conv) Trainium2 Bass kernel.

Problem (hardcoded shapes):
  x:       (16, 128, 128, 128) f32   B, C_in, H, W
  experts: (4, 128, 128, 3, 3) f32   K, C_out, C_in, kh, kw
  bias:    (4, 128) f32              K, C_out
  w1:      (32, 128) f32             HID, C_in
  b1:      (32,) f32
  w2:      (4, 32) f32               K, HID
  b2:      (4,) f32
  out:     (16, 128, 128, 128) f32   B, C_out, H, W  (stride 1, pad 1)

Sharding: data-parallel over batch, 2 samples per core x 8 cores; the tiny
expert/router params are replicated (pre-transposed on the host into the
matmul-friendly layouts -- pure layout prep, all math stays on device).

Per-core dataflow (single pass over x):
  1. x[b] streams in as fp32 16-row slabs on the sync HWDGE ring; DVE chases
     each slab with the channel-sum (exact fp32 mean), ACT casts it into a
     persistent zero-padded fp16 image [C_in, 130, 130].
  2. Router: g = sums/HW -> PE matmuls with pre-transposed w1/w2 -> exp on
     ACT -> softmax denominator + per-k alpha broadcast to 128 partitions via
     tiny PE matmuls with ones/selector stationaries (no cross-partition ops).
  3. weff[b] = sum_k alpha[b,k] * expertT_k on DVE (fp32 accumulate, fp16
     result), layout [C_in, 9, C_out]; beff via fused multiply+accum.
  4. Conv: per 4-row output chunk (512 px), 9 accumulating fp16 matmuls
     (shifted-window taps) into one PSUM bank; ACT evacuates with the
     per-partition beff bias; chunk DMAs out on the ACT ring.
"""

import numpy as np

import concourse.bass as bass
import concourse.mybir as mybir
import concourse.tile as tile
from concourse import bass_utils


def _legalize_waits(nc, keep=1):
    """This container's walrus rejects >1 sync wait per instruction
    (setupSyncWait: "Too many sync wait commands").  Hoist extra waits into
    standalone EventSemaphore wait-nops on the same engine, which is what
    raw-bass wait_ge() emits; ">=" waits commute so order doesn't matter."""
    counter = [0]

    def fix_block(block):
        out, changed = [], False
        for inst in block.instructions:
            si = inst.sync_info
            waits = list(si.on_wait) if si is not None else []
            if len(waits) > keep:
                for w in waits[:-keep]:
                    nm = f"{inst.name}-w{counter[0]}"
                    counter[0] += 1
                    nop = mybir.InstEventSemaphore(name=nm, ins=[], outs=[])
                    nop.engine = inst.engine
                    nop.sync_info = mybir.SyncInfo(on_wait=[w], on_update=[])
                    nc.inst_map[nm] = nop
                    out.append(nop)
                inst.sync_info = mybir.SyncInfo(
                    on_wait=waits[-keep:], on_update=list(si.on_update)
                )
                changed = True
            out.append(inst)
        if changed:
            block.instructions = out
        for sub in getattr(block, "blocks", []) or []:
            fix_block(sub)

    for fn in nc.m.functions:
        for b in fn.blocks:
            fix_block(b)


F32 = mybir.dt.float32
F16 = mybir.dt.float16
AF = mybir.ActivationFunctionType
ALU = mybir.AluOpType

B, CIN, COUT, K, KS, H, W, HID = 16, 128, 128, 4, 3, 128, 128, 32
N_CORES = 8
BPC = B // N_CORES          # samples per core
HP, WP = H + 2, W + 2       # zero-padded image
RPC = 4                     # output rows per chunk
NCHUNK = H // RPC           # 32 chunks per sample
FREE = RPC * W              # 512 = matmul moving free size (one PSUM bank)
NSLAB = 8                   # x-load slabs per sample (16 rows each)
SLAB_ROWS = H // NSLAB
JT = KS * KS                # 9 taps


def build_nc() -> bass.Bass:
    nc = bass.Bass(trn_type="TRN2", target_bir_lowering=False, debug=False)

    x_d = nc.dram_tensor("x", [BPC, CIN, H, W], F32, kind="ExternalInput")
    et_d = nc.dram_tensor("experts_t", [CIN, K, JT, COUT], F32,
                          kind="ExternalInput")
    biast_d = nc.dram_tensor("bias_t", [COUT, K], F32, kind="ExternalInput")
    w1t_d = nc.dram_tensor("w1t", [CIN, HID], F32, kind="ExternalInput")
    b1_d = nc.dram_tensor("b1", [HID], F32, kind="ExternalInput")
    w2t_d = nc.dram_tensor("w2t", [HID, K], F32, kind="ExternalInput")
    b2_d = nc.dram_tensor("b2", [K], F32, kind="ExternalInput")
    y_d = nc.dram_tensor("y", [BPC, COUT, H, W], F32, kind="ExternalOutput")

    with tile.TileContext(nc) as tc:
        with (
            tc.tile_pool(name="singles", bufs=1) as singles,
            tc.tile_pool(name="stage", bufs=3) as stage_pool,
            tc.tile_pool(name="outp", bufs=4) as outp,
            tc.tile_pool(name="pconv", bufs=6, space="PSUM") as pconv,
            tc.tile_pool(name="prt", bufs=2, space="PSUM") as prt,
        ):
            xpads = [None] * BPC
            weffs = [None] * BPC
            alphas = [None] * BPC
            partials_t = [None] * BPC
            beff = singles.tile([COUT, BPC], F32)

            def load_slabs(b, slabs):
                """DMA fp32 16-row slabs of sample b into a staging tile;
                DVE chases each with the channel-sum (fp32, exact mean) and
                ACT casts it into the persistent fp16 padded image (so the
                conv runs 16-bit: single-pass LDWEIGHTS, full-rate
                stream)."""
                if xpads[b] is None:
                    xp = singles.tile([CIN, HP, WP], F16, tag=f"xpad{b}",
                                      name=f"xpad{b}")
                    xpads[b] = xp
                    # zero the 1-px border (rows 0/129, cols 0/129)
                    nc.vector.memset(xp[:, 0, :], 0.0)
                    nc.vector.memset(xp[:, HP - 1, :], 0.0)
                    nc.vector.memset(xp[:, :, 0], 0.0)
                    nc.vector.memset(xp[:, :, WP - 1], 0.0)
                    partials_t[b] = singles.tile(
                        [CIN, NSLAB], F32, tag=f"partials{b}",
                        name=f"partials{b}")
                xp = xpads[b]
                partials = partials_t[b]
                for s in slabs:
                    r0 = s * SLAB_ROWS
                    stage = stage_pool.tile([CIN, SLAB_ROWS, W], F32,
                                            tag="stage")
                    nc.sync.dma_start(
                        out=stage,
                        in_=x_d[b, :, r0:r0 + SLAB_ROWS, :],
                    )
                    nc.vector.tensor_reduce(
                        out=partials[:, s:s + 1],
                        in_=stage,
                        axis=mybir.AxisListType.XY,
                        op=ALU.add,
                    )
                    nc.scalar.activation(
                        out=xp[:, 1 + r0:1 + r0 + SLAB_ROWS, 1:1 + W],
                        in_=stage, func=AF.Copy,
                    )

            def route(b):
                """Router MLP + softmax through broadcast alpha."""
                partials = partials_t[b]
                gT = singles.tile([CIN, 1], F32, tag=f"gT{b}", name=f"gT{b}")
                nc.vector.tensor_reduce(
                    out=gT, in_=partials, axis=mybir.AxisListType.X,
                    op=ALU.add,
                )

                # router MLP + softmax (all tiny, plain fp32)
                h_ps = prt.tile([HID, 1], F32, tag="rt")
                nc.tensor.matmul(h_ps, w1t, gT)
                h_sb = singles.tile([HID, 1], F32, tag=f"h_sb{b}",
                                    name=f"h_sb{b}")
                nc.scalar.activation(out=h_sb, in_=h_ps, func=AF.Relu,
                                     bias=b1t)

                lg_ps = prt.tile([K, 1], F32, tag="rt")
                nc.tensor.matmul(lg_ps, w2t, h_sb)
                # expl = exp(logits + b2); logits are tiny, no max-sub needed
                expl = singles.tile([K, 1], F32, tag=f"expl{b}",
                                    name=f"expl{b}")
                nc.scalar.activation(out=expl, in_=lg_ps, func=AF.Exp,
                                     bias=b2t)

                # softmax denom broadcast to all partitions: ones^T @ expl
                den_ps = prt.tile([128, 1], F32, tag="rt")
                nc.tensor.matmul(den_ps, ones4, expl)
                rS = singles.tile([128, 1], F32, tag=f"rS{b}", name=f"rS{b}")
                nc.vector.reciprocal(out=rS, in_=den_ps)

                # broadcast expl[k] to all partitions: sel_k^T @ expl
                ab_ps = prt.tile([128, K], F32, tag="rt")
                for k in range(K):
                    nc.tensor.matmul(ab_ps[:, k:k + 1], sel[:, k, :], expl)
                alpha = singles.tile([128, K], F32, tag=f"alpha{b}",
                                     name=f"alpha{b}")
                nc.vector.tensor_scalar_mul(alpha, ab_ps, rS)
                alphas[b] = alpha

            def weff_block(b):
                """weff_b = sum_k alpha[k] * expertT_k (fp32 accumulate,
                fp16 result so the conv is all-16-bit); beff_b likewise."""
                alpha = alphas[b]
                wf = singles.tile([CIN, JT, COUT], F16, tag=f"weff{b}",
                                  name=f"weff{b}")
                weffs[b] = wf
                nc.vector.tensor_scalar_mul(
                    wacc, et_flat[:, 0, :], alpha[:, 0:1],
                )
                for k in range(1, K):
                    nc.vector.scalar_tensor_tensor(
                        out=wacc,
                        in0=et_flat[:, k, :],
                        scalar=alpha[:, k:k + 1],
                        in1=wacc,
                        op0=ALU.mult,
                        op1=ALU.add,
                    )
                with nc.allow_low_precision(reason="fp16 conv weights"):
                    nc.vector.tensor_copy(
                        out=wf.rearrange("p j co -> p (j co)"), in_=wacc,
                    )
                btmp = singles.tile([COUT, K], F32, tag="btmp")
                nc.vector.scalar_tensor_tensor(
                    out=btmp,
                    in0=biasT,
                    scalar=1.0,
                    in1=alpha,
                    op0=ALU.mult,
                    op1=ALU.mult,
                    accum_out=beff[:, b:b + 1],
                )

            # phase order = Tile priority order (= DMA lane order): x0 slabs
            # take the fresh HWDGE lanes on the sync ring; the pre-transposed
            # weights ride the ACT ring meanwhile; router-0/weff-0 as soon as
            # x0 lands; x1 + router-1 ABOVE conv-0 so its tiny PE/DVE ops
            # preempt the conv stream the moment their inputs land.
            load_slabs(0, range(NSLAB))

            eT = singles.tile([CIN, K, JT, COUT], F32)
            et_flat = eT.rearrange("p k j co -> p k (j co)")
            nc.scalar.dma_start(out=eT, in_=et_d[:, :, :, :])

            ones4 = singles.tile([K, 128], F32)
            ones4_d = nc.inline_tensor(np.ones((K, 128), np.float32),
                                       name="ones4_const")
            nc.scalar.dma_start(out=ones4, in_=ones4_d[:, :])

            # sel[k', k, m] = 1 if k'==k else 0 (selector stationaries for
            # broadcasting expl[k] to all 128 partitions)
            sel_np = np.zeros((K, K, 128), np.float32)
            for k in range(K):
                sel_np[k, k, :] = 1.0
            sel = singles.tile([K, K, 128], F32)
            sel_d = nc.inline_tensor(sel_np, name="sel_const")
            nc.scalar.dma_start(out=sel, in_=sel_d[:, :, :])

            b1t = singles.tile([HID, 1], F32)
            nc.scalar.dma_start(out=b1t, in_=b1_d[:].unsqueeze(-1))
            b2t = singles.tile([K, 1], F32)
            nc.scalar.dma_start(out=b2t, in_=b2_d[:].unsqueeze(-1))
            w1t = singles.tile([CIN, HID], F32)
            nc.scalar.dma_start(out=w1t, in_=w1t_d[:, :])
            w2t = singles.tile([HID, K], F32)
            nc.scalar.dma_start(out=w2t, in_=w2t_d[:, :])
            biasT = singles.tile([COUT, K], F32)
            nc.scalar.dma_start(out=biasT, in_=biast_d[:, :])

            wacc = singles.tile([CIN, JT * COUT], F32, tag="wacc")

            route(0)
            weff_block(0)
            load_slabs(1, range(NSLAB))
            route(1)
            weff_block(1)

            # ---- conv ---------------------------------------------------
            for b in range(BPC):
                xp = xpads[b]
                wf = weffs[b]
                for hc in range(NCHUNK):
                    ps = pconv.tile([COUT, FREE], F32, tag="ps")
                    for j in range(JT):
                        dy, dx = divmod(j, KS)
                        nc.tensor.matmul(
                            ps,
                            wf[:, j, :],
                            xp[:, RPC * hc + dy:RPC * hc + dy + RPC,
                               dx:dx + W],
                            start=(j == 0),
                            stop=(j == JT - 1),
                        )
                    ot = outp.tile([COUT, FREE], F32, tag="ot")
                    nc.scalar.activation(out=ot, in_=ps, func=AF.Identity,
                                         bias=beff[:, b:b + 1])
                    nc.scalar.dma_start(
                        out=y_d[b, :, RPC * hc:RPC * (hc + 1), :],
                        in_=ot.rearrange("p (r w) -> p r w", w=W),
                    )

    _legalize_waits(nc)
    return nc


_NC_CACHE = None


def get_nc() -> bass.Bass:
    global _NC_CACHE
    if _NC_CACHE is None:
        _NC_CACHE = build_nc()
    return _NC_CACHE


def make_in_maps(inputs: dict[str, np.ndarray]) -> list[dict[str, np.ndarray]]:
    x = np.ascontiguousarray(np.asarray(inputs["x"], dtype=np.float32))
    experts = np.asarray(inputs["experts"], np.float32)
    # host-side layout prep (no math): experts -> lhsT layout [ci, k, j, co];
    # w1 additionally folds the 1/(H*W) mean divisor into its transpose
    et = np.ascontiguousarray(
        experts.reshape(K, COUT, CIN, JT).transpose(2, 0, 3, 1))
    shared = {
        "experts_t": et,
        "bias_t": np.ascontiguousarray(
            np.asarray(inputs["bias"], np.float32).T),
        "w1t": np.ascontiguousarray(
            np.asarray(inputs["w1"], np.float32).T / float(H * W)),
        "b1": np.ascontiguousarray(np.asarray(inputs["b1"], np.float32)),
        "w2t": np.ascontiguousarray(np.asarray(inputs["w2"], np.float32).T),
        "b2": np.ascontiguousarray(np.asarray(inputs["b2"], np.float32)),
    }
    return [
        {"x": x[c * BPC:(c + 1) * BPC], **shared}
        for c in range(N_CORES)
    ]


def kernel(**inputs: np.ndarray) -> np.ndarray:
    nc = get_nc()
    res = bass_utils.run_bass_kernel_spmd(
        nc, make_in_maps(inputs), core_ids=list(range(N_CORES)),
    )
    return np.concatenate(
        [res.results[c]["y"] for c in range(N_CORES)], axis=0)


# revision 31
# speedup vs baseline: 1.0071x; 1.0071x over previous
"""CondConv2d (MoE-routed 3x3 conv) Trainium2 Bass kernel.

Problem (hardcoded shapes):
  x:       (16, 128, 128, 128) f32   B, C_in, H, W
  experts: (4, 128, 128, 3, 3) f32   K, C_out, C_in, kh, kw
  bias:    (4, 128) f32              K, C_out
  w1:      (32, 128) f32             HID, C_in
  b1:      (32,) f32
  w2:      (4, 32) f32               K, HID
  b2:      (4,) f32
  out:     (16, 128, 128, 128) f32   B, C_out, H, W  (stride 1, pad 1)

Sharding: data-parallel over batch, 2 samples per core x 8 cores; the tiny
expert/router params are replicated (pre-transposed on the host into the
matmul-friendly layouts -- pure layout prep, all math stays on device).

Per-core dataflow (single pass over x):
  1. x[b] streams in as fp32 16-row slabs on the sync HWDGE ring; DVE chases
     each slab with the channel-sum (exact fp32 mean), ACT casts it into a
     persistent zero-padded fp16 image [C_in, 130, 130].
  2. Router: g = sums/HW -> PE matmuls with pre-transposed w1/w2 -> exp on
     ACT -> softmax denominator + per-k alpha broadcast to 128 partitions via
     tiny PE matmuls with ones/selector stationaries (no cross-partition ops).
  3. weff[b] = sum_k alpha[b,k] * expertT_k on DVE (fp32 accumulate, fp16
     result), layout [C_in, 9, C_out]; beff via fused multiply+accum.
  4. Conv: per 4-row output chunk (512 px), 9 accumulating fp16 matmuls
     (shifted-window taps) into one PSUM bank; ACT evacuates with the
     per-partition beff bias; chunk DMAs out on the ACT ring.
"""

import numpy as np

import concourse.bass as bass
import concourse.mybir as mybir
import concourse.tile as tile
from concourse import bass_utils


def _legalize_waits(nc, keep=1):
    """This container's walrus rejects >1 sync wait per instruction
    (setupSyncWait: "Too many sync wait commands").  Hoist extra waits into
    standalone EventSemaphore wait-nops on the same engine, which is what
    raw-bass wait_ge() emits; ">=" waits commute so order doesn't matter."""
    counter = [0]

    def fix_block(block):
        out, changed = [], False
        for inst in block.instructions:
            si = inst.sync_info
            waits = list(si.on_wait) if si is not None else []
            if len(waits) > keep:
                for w in waits[:-keep]:
                    nm = f"{inst.name}-w{counter[0]}"
                    counter[0] += 1
                    nop = mybir.InstEventSemaphore(name=nm, ins=[], outs=[])
                    nop.engine = inst.engine
                    nop.sync_info = mybir.SyncInfo(on_wait=[w], on_update=[])
                    nc.inst_map[nm] = nop
                    out.append(nop)
                inst.sync_info = mybir.SyncInfo(
                    on_wait=waits[-keep:], on_update=list(si.on_update)
                )
                changed = True
            out.append(inst)
        if changed:
            block.instructions = out
        for sub in getattr(block, "blocks", []) or []:
            fix_block(sub)

    for fn in nc.m.functions:
        for b in fn.blocks:
            fix_block(b)


F32 = mybir.dt.float32
F16 = mybir.dt.float16
AF = mybir.ActivationFunctionType
ALU = mybir.AluOpType

B, CIN, COUT, K, KS, H, W, HID = 16, 128, 128, 4, 3, 128, 128, 32
N_CORES = 8
BPC = B // N_CORES          # samples per core
HP, WP = H + 2, W + 2       # zero-padded image
RPC = 4                     # output rows per chunk
NCHUNK = H // RPC           # 32 chunks per sample
FREE = RPC * W              # 512 = matmul moving free size (one PSUM bank)
NSLAB = 8                   # x-load slabs per sample (16 rows each)
SLAB_ROWS = H // NSLAB
JT = KS * KS                # 9 taps


def build_nc() -> bass.Bass:
    nc = bass.Bass(trn_type="TRN2", target_bir_lowering=False, debug=False)

    x_d = nc.dram_tensor("x", [BPC, CIN, H, W], F32, kind="ExternalInput")
    et_d = nc.dram_tensor("experts_t", [CIN, K, JT, COUT], F32,
                          kind="ExternalInput")
    biast_d = nc.dram_tensor("bias_t", [COUT, K], F32, kind="ExternalInput")
    w1t_d = nc.dram_tensor("w1t", [CIN, HID], F32, kind="ExternalInput")
    b1_d = nc.dram_tensor("b1", [HID], F32, kind="ExternalInput")
    w2t_d = nc.dram_tensor("w2t", [HID, K], F32, kind="ExternalInput")
    b2_d = nc.dram_tensor("b2", [K], F32, kind="ExternalInput")
    y_d = nc.dram_tensor("y", [BPC, COUT, H, W], F32, kind="ExternalOutput")

    with tile.TileContext(nc) as tc:
        with (
            tc.tile_pool(name="singles", bufs=1) as singles,
            tc.tile_pool(name="stage", bufs=4) as stage_pool,
            tc.tile_pool(name="outp", bufs=4) as outp,
            tc.tile_pool(name="pconv", bufs=6, space="PSUM") as pconv,
            tc.tile_pool(name="prt", bufs=2, space="PSUM") as prt,
        ):
            xpads = [None] * BPC
            weffs = [None] * BPC
            alphas = [None] * BPC
            partials_t = [None] * BPC
            beff = singles.tile([COUT, BPC], F32)

            def load_slabs(b, slabs):
                """DMA fp32 16-row slabs of sample b into a staging tile;
                DVE chases each with the channel-sum (fp32, exact mean) and
                ACT casts it into the persistent fp16 padded image (so the
                conv runs 16-bit: single-pass LDWEIGHTS, full-rate
                stream)."""
                if xpads[b] is None:
                    xp = singles.tile([CIN, HP, WP], F16, tag=f"xpad{b}",
                                      name=f"xpad{b}")
                    xpads[b] = xp
                    # zero the 1-px border (rows 0/129, cols 0/129)
                    nc.vector.memset(xp[:, 0, :], 0.0)
                    nc.vector.memset(xp[:, HP - 1, :], 0.0)
                    nc.vector.memset(xp[:, :, 0], 0.0)
                    nc.vector.memset(xp[:, :, WP - 1], 0.0)
                    partials_t[b] = singles.tile(
                        [CIN, NSLAB], F32, tag=f"partials{b}",
                        name=f"partials{b}")
                xp = xpads[b]
                partials = partials_t[b]
                reduces = []
                for s in slabs:
                    r0 = s * SLAB_ROWS
                    stage = stage_pool.tile([CIN, SLAB_ROWS, W], F32,
                                            tag="stage")
                    # sample 0's tail slabs ride the ACT ring so the head
                    # finishes ~12us sooner than the serial sync ring
                    eng = nc.scalar if (b == 0 and s >= 5) else nc.sync
                    eng.dma_start(
                        out=stage,
                        in_=x_d[b, :, r0:r0 + SLAB_ROWS, :],
                    )
                    red = nc.vector.tensor_reduce(
                        out=partials[:, s:s + 1],
                        in_=stage,
                        axis=mybir.AxisListType.XY,
                        op=ALU.add,
                    )
                    reduces.append(red)
                    nc.scalar.activation(
                        out=xp[:, 1 + r0:1 + r0 + SLAB_ROWS, 1:1 + W],
                        in_=stage, func=AF.Copy,
                    )
                return reduces

            def route(b):
                """Router MLP + softmax through broadcast alpha."""
                partials = partials_t[b]
                gT = singles.tile([CIN, 1], F32, tag=f"gT{b}", name=f"gT{b}")
                nc.vector.tensor_reduce(
                    out=gT, in_=partials, axis=mybir.AxisListType.X,
                    op=ALU.add,
                )

                # router MLP + softmax (all tiny, plain fp32)
                h_ps = prt.tile([HID, 1], F32, tag="rt")
                nc.tensor.matmul(h_ps, w1t, gT)
                h_sb = singles.tile([HID, 1], F32, tag=f"h_sb{b}",
                                    name=f"h_sb{b}")
                nc.scalar.activation(out=h_sb, in_=h_ps, func=AF.Relu,
                                     bias=b1t)

                lg_ps = prt.tile([K, 1], F32, tag="rt")
                nc.tensor.matmul(lg_ps, w2t, h_sb)
                # expl = exp(logits + b2); logits are tiny, no max-sub needed
                expl = singles.tile([K, 1], F32, tag=f"expl{b}",
                                    name=f"expl{b}")
                nc.scalar.activation(out=expl, in_=lg_ps, func=AF.Exp,
                                     bias=b2t)

                # softmax denom broadcast to all partitions: ones^T @ expl
                den_ps = prt.tile([128, 1], F32, tag="rt")
                nc.tensor.matmul(den_ps, ones4, expl)
                rS = singles.tile([128, 1], F32, tag=f"rS{b}", name=f"rS{b}")
                nc.vector.reciprocal(out=rS, in_=den_ps)

                # broadcast expl[k] to all partitions: sel_k^T @ expl
                ab_ps = prt.tile([128, K], F32, tag="rt")
                for k in range(K):
                    nc.tensor.matmul(ab_ps[:, k:k + 1], sel[:, k, :], expl)
                alpha = singles.tile([128, K], F32, tag=f"alpha{b}",
                                     name=f"alpha{b}")
                nc.vector.tensor_scalar_mul(alpha, ab_ps, rS)
                alphas[b] = alpha

            def weff_block(b):
                """weff_b = sum_k alpha[k] * expertT_k (fp32 accumulate,
                fp16 result so the conv is all-16-bit); beff_b likewise."""
                alpha = alphas[b]
                wf = singles.tile([CIN, JT, COUT], F16, tag=f"weff{b}",
                                  name=f"weff{b}")
                weffs[b] = wf
                nc.vector.tensor_scalar_mul(
                    wacc, et_flat[:, 0, :], alpha[:, 0:1],
                )
                for k in range(1, K):
                    nc.vector.scalar_tensor_tensor(
                        out=wacc,
                        in0=et_flat[:, k, :],
                        scalar=alpha[:, k:k + 1],
                        in1=wacc,
                        op0=ALU.mult,
                        op1=ALU.add,
                    )
                with nc.allow_low_precision(reason="fp16 conv weights"):
                    wf_cast = nc.vector.tensor_copy(
                        out=wf.rearrange("p j co -> p (j co)"), in_=wacc,
                    )
                btmp = singles.tile([COUT, K], F32, tag="btmp")
                nc.vector.scalar_tensor_tensor(
                    out=btmp,
                    in0=biasT,
                    scalar=1.0,
                    in1=alpha,
                    op0=ALU.mult,
                    op1=ALU.mult,
                    accum_out=beff[:, b:b + 1],
                )
                return wf_cast

            # phase order = Tile priority order (= DMA lane order): x0 slabs
            # take the fresh HWDGE lanes on the sync ring; the pre-transposed
            # weights ride the ACT ring meanwhile; router-0/weff-0 as soon as
            # x0 lands; x1 + router-1 ABOVE conv-0 so its tiny PE/DVE ops
            # preempt the conv stream the moment their inputs land.
            load_slabs(0, range(NSLAB))

            eT = singles.tile([CIN, K, JT, COUT], F32)
            et_flat = eT.rearrange("p k j co -> p k (j co)")
            nc.scalar.dma_start(out=eT, in_=et_d[:, :, :, :])

            ones4 = singles.tile([K, 128], F32)
            ones4_d = nc.inline_tensor(np.ones((K, 128), np.float32),
                                       name="ones4_const")
            nc.scalar.dma_start(out=ones4, in_=ones4_d[:, :])

            # sel[k', k, m] = 1 if k'==k else 0 (selector stationaries for
            # broadcasting expl[k] to all 128 partitions)
            sel_np = np.zeros((K, K, 128), np.float32)
            for k in range(K):
                sel_np[k, k, :] = 1.0
            sel = singles.tile([K, K, 128], F32)
            sel_d = nc.inline_tensor(sel_np, name="sel_const")
            nc.scalar.dma_start(out=sel, in_=sel_d[:, :, :])

            b1t = singles.tile([HID, 1], F32)
            nc.scalar.dma_start(out=b1t, in_=b1_d[:].unsqueeze(-1))
            b2t = singles.tile([K, 1], F32)
            nc.scalar.dma_start(out=b2t, in_=b2_d[:].unsqueeze(-1))
            w1t = singles.tile([CIN, HID], F32)
            nc.scalar.dma_start(out=w1t, in_=w1t_d[:, :])
            w2t = singles.tile([HID, K], F32)
            nc.scalar.dma_start(out=w2t, in_=w2t_d[:, :])
            biasT = singles.tile([COUT, K], F32)
            nc.scalar.dma_start(out=biasT, in_=biast_d[:, :])

            wacc = singles.tile([CIN, JT * COUT], F32, tag="wacc")

            route(0)
            wf0_cast = weff_block(0)
            x1_reduces = load_slabs(1, range(NSLAB))
            # keep DVE clear for the weff-0 chain: sample 1's channel-sums
            # (which have plenty of slack) are ordered after it
            tile.add_dep_helper(
                x1_reduces[0].ins, wf0_cast.ins, sync=False,
                reason="weff-0 cast preempts x1 reduces on DVE",
            )
            route(1)
            weff_block(1)

            # ---- conv ---------------------------------------------------
            for b in range(BPC):
                xp = xpads[b]
                wf = weffs[b]
                for hc in range(NCHUNK):
                    ps = pconv.tile([COUT, FREE], F32, tag="ps")
                    for j in range(JT):
                        dy, dx = divmod(j, KS)
                        nc.tensor.matmul(
                            ps,
                            wf[:, j, :],
                            xp[:, RPC * hc + dy:RPC * hc + dy + RPC,
                               dx:dx + W],
                            start=(j == 0),
                            stop=(j == JT - 1),
                        )
                    ot = outp.tile([COUT, FREE], F32, tag="ot")
                    nc.scalar.activation(out=ot, in_=ps, func=AF.Identity,
                                         bias=beff[:, b:b + 1])
                    nc.scalar.dma_start(
                        out=y_d[b, :, RPC * hc:RPC * (hc + 1), :],
                        in_=ot.rearrange("p (r w) -> p r w", w=W),
                    )

    _legalize_waits(nc)
    return nc


_NC_CACHE = None


def get_nc() -> bass.Bass:
    global _NC_CACHE
    if _NC_CACHE is None:
        _NC_CACHE = build_nc()
    return _NC_CACHE


def make_in_maps(inputs: dict[str, np.ndarray]) -> list[dict[str, np.ndarray]]:
    x = np.ascontiguousarray(np.asarray(inputs["x"], dtype=np.float32))
    experts = np.asarray(inputs["experts"], np.float32)
    # host-side layout prep (no math): experts -> lhsT layout [ci, k, j, co];
    # w1 additionally folds the 1/(H*W) mean divisor into its transpose
    et = np.ascontiguousarray(
        experts.reshape(K, COUT, CIN, JT).transpose(2, 0, 3, 1))
    shared = {
        "experts_t": et,
        "bias_t": np.ascontiguousarray(
            np.asarray(inputs["bias"], np.float32).T),
        "w1t": np.ascontiguousarray(
            np.asarray(inputs["w1"], np.float32).T / float(H * W)),
        "b1": np.ascontiguousarray(np.asarray(inputs["b1"], np.float32)),
        "w2t": np.ascontiguousarray(np.asarray(inputs["w2"], np.float32).T),
        "b2": np.ascontiguousarray(np.asarray(inputs["b2"], np.float32)),
    }
    return [
        {"x": x[c * BPC:(c + 1) * BPC], **shared}
        for c in range(N_CORES)
    ]


def kernel(**inputs: np.ndarray) -> np.ndarray:
    nc = get_nc()
    res = bass_utils.run_bass_kernel_spmd(
        nc, make_in_maps(inputs), core_ids=list(range(N_CORES)),
    )
    return np.concatenate(
        [res.results[c]["y"] for c in range(N_CORES)], axis=0)
